# revision 2
# baseline (speedup 1.0000x reference)
"""Causal multi-head attention on 8 Trainium2 NeuronCores.

Problem: B=4, S=2048, D=1024, H=16, Dk=64, fp32, causal, all-ones padding mask.

Sharding: core = (batch b, head-group g) on a 4x2 grid. Each core computes the
8 heads of group g for batch b and produces a partial output
x @ (heads g) @ w_o[:, g-slice].T of shape [S, D]; the host sums the two
partials per batch and adds b_o.

Per-core kernel layout (all "T" tensors have the contraction dim on SBUF
partitions):
  - host pre-transposes x and the weight slices (xT, wqT, wkT, wvT, woT)
  - projections (bf16 matmuls, fp32 PSUM accumulation over 8 k-chunks):
      qT, kT in [head-dim, seq] layout (fp32r, scale 1/8 + bias fused in the
      PSUM->SBUF activation copy); v in natural [seq, head-dim] layout (bf16)
      with a ones column appended per head (augmented V)
  - scores computed transposed, st[j, r] = k_j . q_r, as K=64 fp32r matmuls
    into PSUM; only causally-live 512-wide chunks are computed
  - exp on the scalar engine straight out of PSUM into bf16 SBUF tiles; the
    128-wide diagonal block is masked by a multiplicative triangle mask
  - attn@v: oT_aug[c] += v_aug.T @ pexp accumulated over j in PSUM; row 64 is
    the softmax denominator (from the ones column)
  - normalization: recip = exp(-ln(denom)) on the scalar engine, broadcast
    across partitions via a DRAM round-trip DMA, one tensor_mul
  - output projection: fp32r matmuls oT_pair.T @ woT
"""

import numpy as np
from contextlib import ExitStack

import ml_dtypes

P = 128
S = 2048
D = 1024
DK = 64
HLOC = 8          # heads per core
NPAIR = 4         # head pairs (128 rows of qT/kT each)
NCH = 4           # 512-wide q chunks
IC = 8            # 128-deep contraction chunks of D
NT = 16           # 128-tall seq tiles

_PROGRAM_CACHE = {}


def build_program():
    import concourse.bacc as bacc
    import concourse.bass as bass
    import concourse.mybir as mybir
    import concourse.tile as tile

    f32 = mybir.dt.float32
    f32r = mybir.dt.float32r
    bf16 = mybir.dt.bfloat16
    AF = mybir.ActivationFunctionType

    nc = bacc.Bacc("TRN2", target_bir_lowering=False, debug=False)

    xt = nc.dram_tensor("xt", [D, S], bf16, kind="ExternalInput").ap()
    wqt = nc.dram_tensor("wqt", [D, 512], bf16, kind="ExternalInput").ap()
    wkt = nc.dram_tensor("wkt", [D, 512], bf16, kind="ExternalInput").ap()
    wvt = nc.dram_tensor("wvt", [D, 512], bf16, kind="ExternalInput").ap()
    wot = nc.dram_tensor("wot", [512, D], f32r, kind="ExternalInput").ap()
    bqs = nc.dram_tensor("bqs", [P, NPAIR], f32, kind="ExternalInput").ap()
    bks = nc.dram_tensor("bks", [P, NPAIR], f32, kind="ExternalInput").ap()
    bvb = nc.dram_tensor("bvb", [P, 512], f32, kind="ExternalInput").ap()
    out = nc.dram_tensor("out", [S, D], f32, kind="ExternalOutput").ap()
    scr = nc.dram_tensor("scr", [HLOC * NCH, 512], f32, kind="ExternalOutput").ap()

    with tile.TileContext(nc) as tc:
        with ExitStack() as ctx0:
            consts = ctx0.enter_context(tc.tile_pool(name="consts", bufs=1))
            qk = ctx0.enter_context(tc.tile_pool(name="qk", bufs=1))
            vp = ctx0.enter_context(tc.tile_pool(name="vp", bufs=1))
            pexp_pool = ctx0.enter_context(tc.tile_pool(name="pexp", bufs=3))
            oTp_pool = ctx0.enter_context(tc.tile_pool(name="oTp", bufs=1))
            stage_pool = ctx0.enter_context(tc.tile_pool(name="stage", bufs=2))
            norm_pool = ctx0.enter_context(tc.tile_pool(name="norm", bufs=2))

            # constants
            bq_sb = consts.tile([P, NPAIR], f32, tag="bq", name="bq_sb")
            bk_sb = consts.tile([P, NPAIR], f32, tag="bk", name="bk_sb")
            bv_sb = consts.tile([P, 512], f32, tag="bv", name="bv_sb")
            tri = consts.tile([P, P], bf16, tag="tri", name="tri")
            nc.sync.dma_start(bq_sb[:], bqs)
            nc.sync.dma_start(bk_sb[:], bks)
            nc.sync.dma_start(bv_sb[:], bvb)
            # triangle mask: keep col >= row, zero below
            nc.vector.memset(tri[:], 1.0)
            nc.gpsimd.affine_select(
                out=tri[:],
                in_=tri[:],
                compare_op=mybir.AluOpType.is_ge,
                fill=0.0,
                base=0,
                pattern=[[1, P]],
                channel_multiplier=-1,
            )

            # persistent activations
            qT2 = [qk.tile([P, S], f32r, tag=f"qT{m}", name=f"qT{m}") for m in range(NPAIR)]
            kT2 = [qk.tile([P, S], f32r, tag=f"kT{m}", name=f"kT{m}") for m in range(NPAIR)]
            v_sb = [vp.tile([P, HLOC, DK + 1], bf16, tag=f"v{t}", name=f"v{t}") for t in range(NT)]
            oTp = [oTp_pool.tile([P, S], f32r, tag=f"oTp{m}", name=f"oTp{m}") for m in range(NPAIR)]

            with ExitStack() as ctxA:
                oTps = ctxA.enter_context(
                    tc.tile_pool(name="oTps", bufs=4, space="PSUM")
                )
                stps = ctxA.enter_context(
                    tc.tile_pool(name="stps", bufs=1, space="PSUM")
                )

                def attention_head(h):
                    m, half = divmod(h, 2)
                    r0 = 64 * half
                    oT_aug = {}
                    for j in range(NT):
                        c0 = j // 4
                        chunks = list(range(c0, NCH))
                        W = S - P * j
                        pexp_t = pexp_pool.tile([P, W], bf16, tag="pexp", name=f"pexp{h}_{j}")
                        # scores (transposed): st[jj, r] over 512-wide q chunks
                        for tt in range(0, len(chunks), 2):
                            sub = chunks[tt : tt + 2]
                            st = stps.tile([P, 1024], f32, tag="st", name=f"st{h}_{j}_{tt}")
                            for ci, c in enumerate(sub):
                                nc.tensor.matmul(
                                    st[:, 512 * ci : 512 * (ci + 1)],
                                    kT2[m][r0 : r0 + 64, P * j : P * (j + 1)],
                                    qT2[m][r0 : r0 + 64, 512 * c : 512 * (c + 1)],
                                    start=True,
                                    stop=True,
                                )
                            # exp into pexp (skip fully-masked leading cols of
                            # the first chunk)
                            skip = (j % 4) * P if tt == 0 else 0
                            used = 512 * len(sub)
                            pcol = 512 * sub[0] + skip - P * j
                            nc.scalar.activation(
                                pexp_t[:, pcol : pcol + used - skip],
                                st[:, skip:used],
                                AF.Exp,
                            )
                        # mask the diagonal 128-block (strict lower triangle -> 0)
                        nc.vector.tensor_mul(
                            pexp_t[:, 0:P], pexp_t[:, 0:P], tri[:]
                        )
                        # attn@v accumulation
                        for c in chunks:
                            if j == 0:
                                oT_aug[c] = oTps.tile(
                                    [65, 512], f32, tag="oTaug", name=f"oT{h}_{c}"
                                )
                            off = max(0, P * j - 512 * c)
                            pstart = 512 * c + off - P * j
                            nc.tensor.matmul(
                                oT_aug[c][:, off:512],
                                v_sb[j][:, h, :],
                                pexp_t[:, pstart : pstart + 512 - off],
                                start=(j == 0),
                                stop=(j == 4 * c + 3),
                            )
                    # normalize and write into the pair tile
                    for c in range(NCH):
                        t1 = norm_pool.tile([P, 512], f32, tag="t1", name=f"t1_{h}_{c}")
                        nc.scalar.activation(
                            t1[64:65, :], oT_aug[c][64:65, :], AF.Ln
                        )
                        r1 = norm_pool.tile([P, 512], f32, tag="r1", name=f"r1_{h}_{c}")
                        nc.scalar.activation(
                            r1[64:65, :], t1[64:65, :], AF.Exp, bias=0.0, scale=-1.0
                        )
                        idx = h * NCH + c
                        nc.sync.dma_start(scr[idx : idx + 1, :], r1[64:65, :])
                        rb = norm_pool.tile([64, 512], f32, tag="rb", name=f"rb_{h}_{c}")
                        bcast_src = bass.AP(
                            tensor=scr.tensor,
                            offset=scr.offset + idx * 512,
                            ap=[[0, 64], [1, 512]],
                        )
                        nc.sync.dma_start(rb[:], bcast_src)
                        if half == 0:
                            nc.vector.tensor_mul(
                                oTp[m][0:64, 512 * c : 512 * (c + 1)],
                                oT_aug[c][0:64, :],
                                rb[:],
                            )
                        else:
                            stg = stage_pool.tile([64, 512], f32r, tag="stg", name=f"stg_{h}_{c}")
                            nc.vector.tensor_mul(stg[:], oT_aug[c][0:64, :], rb[:])
                            nc.sync.dma_start(
                                oTp[m][64:P, 512 * c : 512 * (c + 1)], stg[:]
                            )

                with ExitStack() as ctxP:
                    xpool = ctxP.enter_context(tc.tile_pool(name="xpool", bufs=1))
                    wvpool = ctxP.enter_context(tc.tile_pool(name="wvpool", bufs=1))
                    wqkpool = ctxP.enter_context(
                        tc.tile_pool(name="wqkpool", bufs=4)
                    )
                    pps = ctxP.enter_context(
                        tc.tile_pool(name="pps", bufs=2, space="PSUM")
                    )

                    xT_sb = xpool.tile([P, IC, S], bf16, tag="xT", name="xT_sb")
                    for ic in range(IC):
                        nc.sync.dma_start(
                            xT_sb[:, ic, :], xt[P * ic : P * (ic + 1), :]
                        )
                    wv_sb = wvpool.tile([P, IC, 512], bf16, tag="wv", name="wv_sb")
                    for ic in range(IC):
                        nc.sync.dma_start(
                            wv_sb[:, ic, :], wvt[P * ic : P * (ic + 1), :]
                        )

                    # V projection (natural layout) + bias + ones column
                    for t in range(NT):
                        psv = pps.tile([P, 512], f32, tag="pps", name="ppst")
                        for ic in range(IC):
                            nc.tensor.matmul(
                                psv[:],
                                xT_sb[:, ic, P * t : P * (t + 1)],
                                wv_sb[:, ic, :],
                                start=(ic == 0),
                                stop=(ic == IC - 1),
                            )
                        nc.vector.tensor_add(
                            v_sb[t][:, :, 0:DK],
                            psv[:].rearrange("p (h d) -> p h d", d=DK),
                            bv_sb[:].rearrange("p (h d) -> p h d", d=DK),
                        )
                        nc.vector.memset(v_sb[t][:, :, DK : DK + 1], 1.0)

                    def project_pair(m):
                        wq_m = wqkpool.tile([P, IC, P], bf16, tag="wqk", name=f"wqk{m}")
                        wk_m = wqkpool.tile([P, IC, P], bf16, tag="wqk", name=f"wqk{m}")
                        nc.sync.dma_start(
                            wq_m[:],
                            wqt.rearrange("(ic p) o -> p ic o", p=P)[
                                :, :, P * m : P * (m + 1)
                            ],
                        )
                        nc.sync.dma_start(
                            wk_m[:],
                            wkt.rearrange("(ic p) o -> p ic o", p=P)[
                                :, :, P * m : P * (m + 1)
                            ],
                        )
                        for c in range(NCH):
                            psq = pps.tile([P, 512], f32, tag="pps", name="ppst")
                            for ic in range(IC):
                                nc.tensor.matmul(
                                    psq[:],
                                    wq_m[:, ic, :],
                                    xT_sb[:, ic, 512 * c : 512 * (c + 1)],
                                    start=(ic == 0),
                                    stop=(ic == IC - 1),
                                )
                            nc.scalar.activation(
                                qT2[m][:, 512 * c : 512 * (c + 1)],
                                psq[:],
                                AF.Identity,
                                bias=bq_sb[:, m : m + 1],
                                scale=0.125,
                            )
                            psk = pps.tile([P, 512], f32, tag="pps", name="ppst")
                            for ic in range(IC):
                                nc.tensor.matmul(
                                    psk[:],
                                    wk_m[:, ic, :],
                                    xT_sb[:, ic, 512 * c : 512 * (c + 1)],
                                    start=(ic == 0),
                                    stop=(ic == IC - 1),
                                )
                            nc.scalar.activation(
                                kT2[m][:, 512 * c : 512 * (c + 1)],
                                psk[:],
                                AF.Identity,
                                bias=bk_sb[:, m : m + 1],
                                scale=1.0,
                            )

                    project_pair(0)
                    for m in range(NPAIR):
                        attention_head(2 * m)
                        attention_head(2 * m + 1)
                        if m + 1 < NPAIR:
                            project_pair(m + 1)

            # output projection
            wo_pool = tc.tile_pool(name="wo", bufs=1)
            fst_pool = tc.tile_pool(name="fst", bufs=3)
            fin_ps = tc.tile_pool(name="finps", bufs=3, space="PSUM")
            with wo_pool as wop, fst_pool as fsp, fin_ps as fps:
                woT_sb = wop.tile([P, NPAIR, D], f32r, tag="woT", name="woT_sb")
                nc.sync.dma_start(
                    woT_sb[:], wot.rearrange("(m p) o -> p m o", p=P)
                )
                for t in range(NT):
                    for n in range(2):
                        psf = fps.tile([P, 512], f32, tag="fin", name=f"fin{t}_{n}")
                        for m in range(NPAIR):
                            nc.tensor.matmul(
                                psf[:],
                                oTp[m][:, P * t : P * (t + 1)],
                                woT_sb[:, m, 512 * n : 512 * (n + 1)],
                                start=(m == 0),
                                stop=(m == NPAIR - 1),
                            )
                        fstg = fsp.tile([P, 512], f32, tag="fstg", name=f"fstg{t}_{n}")
                        nc.any.tensor_copy(fstg[:], psf[:])
                        nc.sync.dma_start(
                            out[P * t : P * (t + 1), 512 * n : 512 * (n + 1)],
                            fstg[:],
                        )

    nc.compile()
    return nc


def get_program():
    if "nc" not in _PROGRAM_CACHE:
        _PROGRAM_CACHE["nc"] = build_program()
    return _PROGRAM_CACHE["nc"]


def make_in_maps(x, w_q, b_q, w_k, b_k, w_v, b_v, w_o):
    bf = ml_dtypes.bfloat16
    x = np.asarray(x, np.float32)
    w_q = np.asarray(w_q, np.float32)
    w_k = np.asarray(w_k, np.float32)
    w_v = np.asarray(w_v, np.float32)
    w_o = np.asarray(w_o, np.float32)
    b_q = np.asarray(b_q, np.float32)
    b_k = np.asarray(b_k, np.float32)
    b_v = np.asarray(b_v, np.float32)
    in_maps = []
    for core in range(8):
        b, g = divmod(core, 2)
        sl = slice(512 * g, 512 * (g + 1))
        in_maps.append(
            {
                "xt": np.ascontiguousarray(x[b].T).astype(bf),
                "wqt": np.ascontiguousarray(w_q[sl].T).astype(bf),
                "wkt": np.ascontiguousarray(w_k[sl].T).astype(bf),
                "wvt": np.ascontiguousarray(w_v[sl].T).astype(bf),
                "wot": np.ascontiguousarray(w_o[:, sl].T),
                "bqs": np.ascontiguousarray((b_q[sl] * 0.125).reshape(NPAIR, P).T),
                "bks": np.ascontiguousarray(b_k[sl].reshape(NPAIR, P).T),
                "bvb": np.ascontiguousarray(np.tile(b_v[sl][None, :], (P, 1))),
            }
        )
    return in_maps


def kernel(x, mask, w_q, b_q, w_k, b_k, w_v, b_v, w_o, b_o):
    from concourse.bass_utils import run_bass_kernel_spmd

    nc = get_program()
    in_maps = make_in_maps(x, w_q, b_q, w_k, b_k, w_v, b_v, w_o)
    res = run_bass_kernel_spmd(nc, in_maps, core_ids=list(range(8)), trace=False)
    b_o = np.asarray(b_o, np.float32)
    outs = []
    for b in range(4):
        outs.append(
            res.results[2 * b]["out"] + res.results[2 * b + 1]["out"] + b_o[None, :]
        )
    return np.stack(outs).astype(np.float32)


# revision 8
# speedup vs baseline: 95.7648x; 95.7648x over previous
"""Causal multi-head attention on 8 Trainium2 NeuronCores.

Problem: B=4, S=2048, D=1024, H=16, Dk=64, fp32, causal, all-ones padding mask.

Sharding: core = (batch b, head-group g) on a 4x2 grid. Each core computes the
8 heads of group g for batch b and produces a partial output
x @ (heads g) @ w_o[:, g-slice].T of shape [S, D]; the host sums the two
partials per batch and adds b_o.

Per-core kernel layout (all "T" tensors have the contraction dim on SBUF
partitions):
  - host pre-transposes x and the weight slices (xT, wqT, wkT, wvT, woT)
  - projections (bf16 matmuls, fp32 PSUM accumulation over 8 k-chunks):
      qT, kT in [head-dim, seq] layout (fp32r, scale 1/8 + bias fused in the
      PSUM->SBUF activation copy); v in natural [seq, head-dim] layout (bf16)
      with a ones column appended per head (augmented V)
  - attention runs per head-pair, q-chunk outer / k-tile inner; the two
    heads' score matmuls (K=64 fp32r) land in disjoint PE row groups
    (partitions 0:64 / 64:128) and overlap on hardware
  - one exp per (pair, chunk, k-tile) over both heads' scores straight out
    of PSUM into bf16 SBUF; the diagonal 128-block is masked by a
    multiplicative triangle mask
  - attn@v: oT_aug += v_aug.T @ pexp accumulated over k-tiles in PSUM;
    row 64 is the softmax denominator (from the ones column)
  - normalization: denominator -> DRAM, spread over 64 partitions, DVE
    reciprocal, back to DRAM, partition-broadcast DMA, one tensor_mul
  - output projection: fp32r matmuls oT_pair.T @ woT
"""

import numpy as np
from contextlib import ExitStack

import ml_dtypes

P = 128
S = 2048
D = 1024
DK = 64
HLOC = 8          # heads per core
NPAIR = 4         # head pairs (128 rows of qT/kT each)
NCH = 4           # 512-wide q chunks
IC = 8            # 128-deep contraction chunks of D
NT = 16           # 128-tall seq tiles

_PROGRAM_CACHE = {}


def build_program(reps=1):
    import concourse.bacc as bacc
    import concourse.bass as bass
    import concourse.mybir as mybir
    import concourse.tile as tile

    f32 = mybir.dt.float32
    f32r = mybir.dt.float32r
    bf16 = mybir.dt.bfloat16
    AF = mybir.ActivationFunctionType

    nc = bacc.Bacc("TRN2", target_bir_lowering=False, debug=False)

    xt = nc.dram_tensor("xt", [D, S], bf16, kind="ExternalInput").ap()
    wqt = nc.dram_tensor("wqt", [D, 512], bf16, kind="ExternalInput").ap()
    wkt = nc.dram_tensor("wkt", [D, 512], bf16, kind="ExternalInput").ap()
    wvt = nc.dram_tensor("wvt", [D, 512], bf16, kind="ExternalInput").ap()
    wot = nc.dram_tensor("wot", [512, D], f32r, kind="ExternalInput").ap()
    bqs = nc.dram_tensor("bqs", [P, NPAIR], f32, kind="ExternalInput").ap()
    bks = nc.dram_tensor("bks", [P, NPAIR], f32, kind="ExternalInput").ap()
    bvb = nc.dram_tensor("bvb", [P, 512], f32, kind="ExternalInput").ap()
    out = nc.dram_tensor("out", [S, D], f32, kind="ExternalOutput").ap()
    # scratch rows: [idx] raw denominator, [32+idx] reciprocal
    scr = nc.dram_tensor("scr", [2 * HLOC * NCH, 512], f32, kind="ExternalOutput").ap()

    with tile.TileContext(nc) as tc:
      for _rep in range(reps):
        sfx = f"_r{_rep}" if reps > 1 else ""
        with ExitStack() as ctx0:
            consts = ctx0.enter_context(tc.tile_pool(name="consts" + sfx, bufs=1))
            vp = ctx0.enter_context(tc.tile_pool(name="vp" + sfx, bufs=1))
            pexp_pool = ctx0.enter_context(tc.tile_pool(name="pexp" + sfx, bufs=4))
            oTp_pool = ctx0.enter_context(tc.tile_pool(name="oTp" + sfx, bufs=1))
            stage_pool = ctx0.enter_context(tc.tile_pool(name="stage" + sfx, bufs=2))
            norm_pool = ctx0.enter_context(tc.tile_pool(name="norm" + sfx, bufs=4))

            # constants
            bq_sb = consts.tile([P, NPAIR], f32, tag="bq", name="bq_sb" + sfx)
            bk_sb = consts.tile([P, NPAIR], f32, tag="bk", name="bk_sb" + sfx)
            bv_sb = consts.tile([P, 512], f32, tag="bv", name="bv_sb" + sfx)
            tri = consts.tile([P, P], bf16, tag="tri", name="tri" + sfx)
            nc.sync.dma_start(bq_sb[:], bqs)
            nc.sync.dma_start(bk_sb[:], bks)
            nc.sync.dma_start(bv_sb[:], bvb)
            # triangle mask: keep col >= row, zero below
            nc.vector.memset(tri[:], 1.0)
            nc.gpsimd.affine_select(
                out=tri[:],
                in_=tri[:],
                compare_op=mybir.AluOpType.is_ge,
                fill=0.0,
                base=0,
                pattern=[[1, P]],
                channel_multiplier=-1,
            )

            # persistent activations: qT/kT rotate through a 2-deep pool
            # (pair m+1 reuses pair m-1's slot once its attention is done)
            qk = ctx0.enter_context(tc.tile_pool(name="qk" + sfx, bufs=2))
            qT2, kT2 = {}, {}
            v_sb = [
                vp.tile([P, HLOC, DK + 1], bf16, tag=f"v{t}", name=f"v{t}" + sfx)
                for t in range(NT)
            ]
            oTp = [
                oTp_pool.tile([P, S], f32r, tag=f"oTp{m}", name=f"oTp{m}" + sfx)
                for m in range(NPAIR)
            ]
            # output-projection weights: loaded up front so the final phase
            # never waits on this DMA
            woT_sb = consts.tile([P, NPAIR, D], f32r, tag="woT", name="woT_sb" + sfx)
            nc.sync.dma_start(woT_sb[:], wot.rearrange("(m p) o -> p m o", p=P))

            with ExitStack() as ctxA:
                oTps = ctxA.enter_context(
                    tc.tile_pool(name="oTps" + sfx, bufs=3, space="PSUM")
                )
                stps = ctxA.enter_context(
                    tc.tile_pool(name="stps" + sfx, bufs=2, space="PSUM")
                )

                def normalize(h, c, oT_aug):
                    m, half = divmod(h, 2)
                    idx = h * NCH + c
                    t1 = norm_pool.tile([P, 512], f32, tag="t1", name=f"t1_{h}_{c}" + sfx)
                    nc.vector.tensor_copy(t1[64:65, :], oT_aug[64:65, :])
                    oT_un = norm_pool.tile(
                        [64, 512], f32, tag="oT_un", name=f"oTun_{h}_{c}" + sfx
                    )
                    nc.vector.tensor_copy(oT_un[:], oT_aug[0:64, :])
                    # denom row -> DRAM, spread over 64 partitions, DVE
                    # reciprocal, back to DRAM, broadcast to [64, 512]
                    nc.sync.dma_start(scr[idx : idx + 1, :], t1[64:65, :])
                    spread = norm_pool.tile(
                        [64, 8], f32, tag="spread", name=f"spread_{h}_{c}" + sfx
                    )
                    spread_src = bass.AP(
                        tensor=scr.tensor,
                        offset=scr.offset + idx * 512,
                        ap=[[8, 64], [1, 8]],
                    )
                    nc.sync.dma_start(spread[:], spread_src)
                    rspread = norm_pool.tile(
                        [64, 8], f32, tag="rspread", name=f"rspread_{h}_{c}" + sfx
                    )
                    nc.vector.reciprocal(rspread[:], spread[:])
                    nc.sync.dma_start(
                        bass.AP(
                            tensor=scr.tensor,
                            offset=scr.offset + (32 + idx) * 512,
                            ap=[[8, 64], [1, 8]],
                        ),
                        rspread[:],
                    )
                    rb = norm_pool.tile([64, 512], f32, tag="rb", name=f"rb_{h}_{c}" + sfx)
                    bcast_src = bass.AP(
                        tensor=scr.tensor,
                        offset=scr.offset + (32 + idx) * 512,
                        ap=[[0, 64], [1, 512]],
                    )
                    nc.sync.dma_start(rb[:], bcast_src)
                    if half == 0:
                        nc.vector.tensor_mul(
                            oTp[m][0:64, 512 * c : 512 * (c + 1)],
                            oT_un[:],
                            rb[:],
                        )
                    else:
                        stg = stage_pool.tile(
                            [64, 512], f32r, tag="stg", name=f"stg_{h}_{c}" + sfx
                        )
                        nc.vector.tensor_mul(stg[:], oT_un[:], rb[:])
                        nc.sync.dma_start(
                            oTp[m][64:P, 512 * c : 512 * (c + 1)], stg[:]
                        )

                def attention_pair(m):
                    """Heads 2m (qT/kT rows 0:64) and 2m+1 (rows 64:128),
                    c-outer / j-inner.  The two heads' score matmuls go to
                    disjoint PE row groups and run concurrently."""
                    ha, hb = 2 * m, 2 * m + 1
                    for c in range(NCH):
                        oT_a = oTps.tile(
                            [65, 512], f32, tag="oTaug", name=f"oTa{m}_{c}" + sfx
                        )
                        oT_b = oTps.tile(
                            [65, 512], f32, tag="oTaug", name=f"oTb{m}_{c}" + sfx
                        )
                        for j in range(4 * c + 4):
                            off = max(0, P * j - 512 * c)
                            w = 512 - off
                            st_ab = stps.tile(
                                [P, 1024], f32, tag="st", name=f"st{m}_{c}_{j}" + sfx
                            )
                            nc.tensor.matmul(
                                st_ab[:, off:512],
                                kT2[m][0:64, P * j : P * (j + 1)],
                                qT2[m][0:64, 512 * c + off : 512 * (c + 1)],
                                start=True,
                                stop=True,
                            )
                            nc.tensor.matmul(
                                st_ab[:, 512 + off : 1024],
                                kT2[m][64:P, P * j : P * (j + 1)],
                                qT2[m][64:P, 512 * c + off : 512 * (c + 1)],
                                start=True,
                                stop=True,
                            )
                            pexp2 = pexp_pool.tile(
                                [P, 2, 512], bf16, tag="pexp",
                                name=f"pexp{m}_{c}_{j}" + sfx,
                            )
                            nc.scalar.activation(
                                pexp2[:, :, 0:w],
                                st_ab[:].rearrange("p (two x) -> p two x", two=2)[
                                    :, :, off:512
                                ],
                                AF.Exp,
                            )
                            if j >= 4 * c:
                                # diagonal 128-block: strict lower triangle -> 0
                                nc.vector.tensor_mul(
                                    pexp2[:, :, 0:P],
                                    pexp2[:, :, 0:P],
                                    tri[:, None, :].to_broadcast((P, 2, P)),
                                )
                            nc.tensor.matmul(
                                oT_a[:, off:512],
                                v_sb[j][:, ha, :],
                                pexp2[:, 0, 0:w],
                                start=(j == 0),
                                stop=(j == 4 * c + 3),
                            )
                            nc.tensor.matmul(
                                oT_b[:, off:512],
                                v_sb[j][:, hb, :],
                                pexp2[:, 1, 0:w],
                                start=(j == 0),
                                stop=(j == 4 * c + 3),
                            )
                        normalize(ha, c, oT_a)
                        normalize(hb, c, oT_b)

                with ExitStack() as ctxP:
                    xpool = ctxP.enter_context(
                        tc.tile_pool(name="xpool" + sfx, bufs=1)
                    )
                    wvpool = ctxP.enter_context(
                        tc.tile_pool(name="wvpool" + sfx, bufs=1)
                    )
                    wqkpool = ctxP.enter_context(
                        tc.tile_pool(name="wqkpool" + sfx, bufs=4)
                    )
                    pps = ctxP.enter_context(
                        tc.tile_pool(name="pps" + sfx, bufs=1, space="PSUM")
                    )

                    wv_sb = wvpool.tile(
                        [P, IC, 512], bf16, tag="wv", name="wv_sb" + sfx
                    )
                    for ic in range(IC):
                        nc.sync.dma_start(
                            wv_sb[:, ic, :], wvt[P * ic : P * (ic + 1), :]
                        )
                    xT_sb = xpool.tile([P, IC, S], bf16, tag="xT", name="xT_sb" + sfx)
                    for ic in range(IC):
                        nc.sync.dma_start(
                            xT_sb[:, ic, :], xt[P * ic : P * (ic + 1), :]
                        )

                    # V projection (natural layout) + bias + ones column
                    for t in range(NT):
                        psv = stps.tile([P, 1024], f32, tag="st", name=f"psv{t}" + sfx)
                        for ic in range(IC):
                            nc.tensor.matmul(
                                psv[:, 0:512],
                                xT_sb[:, ic, P * t : P * (t + 1)],
                                wv_sb[:, ic, :],
                                start=(ic == 0),
                                stop=(ic == IC - 1),
                            )
                        nc.vector.tensor_add(
                            v_sb[t][:, :, 0:DK],
                            psv[:, 0:512].rearrange("p (h d) -> p h d", d=DK),
                            bv_sb[:].rearrange("p (h d) -> p h d", d=DK),
                        )
                        nc.vector.memset(v_sb[t][:, :, DK : DK + 1], 1.0)

                    def project_pair(m):
                        qT2[m] = qk.tile(
                            [P, S], f32r, tag="qT", name=f"qT{m}" + sfx
                        )
                        kT2[m] = qk.tile(
                            [P, S], f32r, tag="kT", name=f"kT{m}" + sfx
                        )
                        wq_m = wqkpool.tile(
                            [P, IC, P], bf16, tag="wqk", name=f"wq{m}" + sfx
                        )
                        wk_m = wqkpool.tile(
                            [P, IC, P], bf16, tag="wqk", name=f"wk{m}" + sfx
                        )
                        nc.sync.dma_start(
                            wq_m[:],
                            wqt.rearrange("(ic p) o -> p ic o", p=P)[
                                :, :, P * m : P * (m + 1)
                            ],
                        )
                        nc.sync.dma_start(
                            wk_m[:],
                            wkt.rearrange("(ic p) o -> p ic o", p=P)[
                                :, :, P * m : P * (m + 1)
                            ],
                        )
                        for c in range(NCH):
                            psq = pps.tile(
                                [P, 512], f32, tag="pps", name=f"psq{m}_{c}" + sfx
                            )
                            for ic in range(IC):
                                nc.tensor.matmul(
                                    psq[:],
                                    wq_m[:, ic, :],
                                    xT_sb[:, ic, 512 * c : 512 * (c + 1)],
                                    start=(ic == 0),
                                    stop=(ic == IC - 1),
                                )
                            nc.scalar.activation(
                                qT2[m][:, 512 * c : 512 * (c + 1)],
                                psq[:],
                                AF.Identity,
                                bias=bq_sb[:, m : m + 1],
                                scale=0.125,
                            )
                            psk = pps.tile(
                                [P, 512], f32, tag="pps", name=f"psk{m}_{c}" + sfx
                            )
                            for ic in range(IC):
                                nc.tensor.matmul(
                                    psk[:],
                                    wk_m[:, ic, :],
                                    xT_sb[:, ic, 512 * c : 512 * (c + 1)],
                                    start=(ic == 0),
                                    stop=(ic == IC - 1),
                                )
                            nc.scalar.activation(
                                kT2[m][:, 512 * c : 512 * (c + 1)],
                                psk[:],
                                AF.Identity,
                                bias=bk_sb[:, m : m + 1],
                                scale=1.0,
                            )

                    project_pair(0)
                    for m in range(NPAIR):
                        attention_pair(m)
                        if m + 1 < NPAIR:
                            project_pair(m + 1)

            # output projection
            fst_pool = tc.tile_pool(name="fst" + sfx, bufs=4)
            fin_ps = tc.tile_pool(name="finps" + sfx, bufs=6, space="PSUM")
            with fst_pool as fsp, fin_ps as fps:
                for t in range(NT):
                    for n in range(2):
                        psf = fps.tile(
                            [P, 512], f32, tag="fin", name=f"fin{t}_{n}" + sfx
                        )
                        for m in range(NPAIR):
                            nc.tensor.matmul(
                                psf[:],
                                oTp[m][:, P * t : P * (t + 1)],
                                woT_sb[:, m, 512 * n : 512 * (n + 1)],
                                start=(m == 0),
                                stop=(m == NPAIR - 1),
                            )
                        fstg = fsp.tile(
                            [P, 512], f32, tag="fstg", name=f"fstg{t}_{n}" + sfx
                        )
                        nc.any.tensor_copy(fstg[:], psf[:])
                        nc.sync.dma_start(
                            out[P * t : P * (t + 1), 512 * n : 512 * (n + 1)],
                            fstg[:],
                        )

    nc.compile()
    return nc


def get_program(reps=1):
    if reps not in _PROGRAM_CACHE:
        _PROGRAM_CACHE[reps] = build_program(reps)
    return _PROGRAM_CACHE[reps]


def make_in_maps(x, w_q, b_q, w_k, b_k, w_v, b_v, w_o):
    bf = ml_dtypes.bfloat16
    x = np.asarray(x, np.float32)
    w_q = np.asarray(w_q, np.float32)
    w_k = np.asarray(w_k, np.float32)
    w_v = np.asarray(w_v, np.float32)
    w_o = np.asarray(w_o, np.float32)
    b_q = np.asarray(b_q, np.float32)
    b_k = np.asarray(b_k, np.float32)
    b_v = np.asarray(b_v, np.float32)
    in_maps = []
    for core in range(8):
        b, g = divmod(core, 2)
        sl = slice(512 * g, 512 * (g + 1))
        in_maps.append(
            {
                "xt": np.ascontiguousarray(x[b].T).astype(bf),
                "wqt": np.ascontiguousarray(w_q[sl].T).astype(bf),
                "wkt": np.ascontiguousarray(w_k[sl].T).astype(bf),
                "wvt": np.ascontiguousarray(w_v[sl].T).astype(bf),
                "wot": np.ascontiguousarray(w_o[:, sl].T),
                "bqs": np.ascontiguousarray((b_q[sl] * 0.125).reshape(NPAIR, P).T),
                "bks": np.ascontiguousarray(b_k[sl].reshape(NPAIR, P).T),
                "bvb": np.ascontiguousarray(np.tile(b_v[sl][None, :], (P, 1))),
            }
        )
    return in_maps


def kernel(x, mask, w_q, b_q, w_k, b_k, w_v, b_v, w_o, b_o):
    from concourse.bass_utils import run_bass_kernel_spmd

    nc = get_program()
    in_maps = make_in_maps(x, w_q, b_q, w_k, b_k, w_v, b_v, w_o)
    res = run_bass_kernel_spmd(nc, in_maps, core_ids=list(range(8)), trace=False)
    b_o = np.asarray(b_o, np.float32)
    outs = []
    for b in range(4):
        outs.append(
            res.results[2 * b]["out"] + res.results[2 * b + 1]["out"] + b_o[None, :]
        )
    return np.stack(outs).astype(np.float32)


# revision 9
# speedup vs baseline: 369.0060x; 3.8533x over previous
"""Causal multi-head attention on 8 Trainium2 NeuronCores.

Problem: B=4, S=2048, D=1024, H=16, Dk=64, fp32, causal, all-ones padding mask.

Sharding: core = (batch b, head-group g) on a 4x2 grid. Each core computes the
8 heads of group g for batch b and produces a partial output
x @ (heads g) @ w_o[:, g-slice].T of shape [S, D]; the host sums the two
partials per batch and adds b_o.

Per-core kernel layout (all "T" tensors have the contraction dim on SBUF
partitions):
  - host pre-transposes x and the weight slices (xT, wqT, wkT, wvT, woT)
  - projections (bf16 matmuls, fp32 PSUM accumulation over 8 k-chunks):
      qT, kT in [head-dim, seq] layout (fp32r, scale 1/8 + bias fused in the
      PSUM->SBUF activation copy); v in natural [seq, head-dim] layout (bf16)
      with a ones column appended per head (augmented V)
  - attention runs per head-pair, q-chunk outer / k-tile inner; the two
    heads' score matmuls (K=64 fp32r) land in disjoint PE row groups
    (partitions 0:64 / 64:128) and overlap on hardware
  - one exp per (pair, chunk, k-tile) over both heads' scores straight out
    of PSUM into bf16 SBUF; the diagonal 128-block is masked by a
    multiplicative triangle mask
  - attn@v: oT_aug += v_aug.T @ pexp accumulated over k-tiles in PSUM;
    row 64 is the softmax denominator (from the ones column)
  - normalization: denominator -> DRAM, spread over 64 partitions, DVE
    reciprocal, back to DRAM, partition-broadcast DMA, one tensor_mul
  - output projection: fp32r matmuls oT_pair.T @ woT
"""

import numpy as np
from contextlib import ExitStack

import ml_dtypes

P = 128
S = 2048
D = 1024
DK = 64
HLOC = 8          # heads per core
NPAIR = 4         # head pairs (128 rows of qT/kT each)
NCH = 4           # 512-wide q chunks
IC = 8            # 128-deep contraction chunks of D
NT = 16           # 128-tall seq tiles

_PROGRAM_CACHE = {}


def build_program(reps=1):
    import concourse.bacc as bacc
    import concourse.bass as bass
    import concourse.mybir as mybir
    import concourse.tile as tile

    f32 = mybir.dt.float32
    f32r = mybir.dt.float32r
    bf16 = mybir.dt.bfloat16
    AF = mybir.ActivationFunctionType

    nc = bacc.Bacc("TRN2", target_bir_lowering=False, debug=False)

    xt = nc.dram_tensor("xt", [D, S], bf16, kind="ExternalInput").ap()
    wqt = nc.dram_tensor("wqt", [D, 512], bf16, kind="ExternalInput").ap()
    wkt = nc.dram_tensor("wkt", [D, 512], bf16, kind="ExternalInput").ap()
    wvt = nc.dram_tensor("wvt", [D, 512], bf16, kind="ExternalInput").ap()
    wot = nc.dram_tensor("wot", [512, D], f32r, kind="ExternalInput").ap()
    bqs = nc.dram_tensor("bqs", [P, NPAIR], f32, kind="ExternalInput").ap()
    bks = nc.dram_tensor("bks", [P, NPAIR], f32, kind="ExternalInput").ap()
    bvb = nc.dram_tensor("bvb", [P, 512], f32, kind="ExternalInput").ap()
    out = nc.dram_tensor("out", [S, D], f32, kind="ExternalOutput").ap()
    # scratch rows: [idx] raw denominator, [32+idx] reciprocal
    scr = nc.dram_tensor("scr", [2 * HLOC * NCH, 512], f32, kind="ExternalOutput").ap()

    with tile.TileContext(nc) as tc:
      for _rep in range(reps):
        sfx = f"_r{_rep}" if reps > 1 else ""
        with ExitStack() as ctx0:
            consts = ctx0.enter_context(tc.tile_pool(name="consts" + sfx, bufs=1))
            vp = ctx0.enter_context(tc.tile_pool(name="vp" + sfx, bufs=1))
            pexp_pool = ctx0.enter_context(tc.tile_pool(name="pexp" + sfx, bufs=6))
            oTp_pool = ctx0.enter_context(tc.tile_pool(name="oTp" + sfx, bufs=1))
            stage_pool = ctx0.enter_context(tc.tile_pool(name="stage" + sfx, bufs=2))
            norm_pool = ctx0.enter_context(tc.tile_pool(name="norm" + sfx, bufs=4))

            # constants
            bq_sb = consts.tile([P, NPAIR], f32, tag="bq", name="bq_sb" + sfx)
            bk_sb = consts.tile([P, NPAIR], f32, tag="bk", name="bk_sb" + sfx)
            bv_sb = consts.tile([P, 512], f32, tag="bv", name="bv_sb" + sfx)
            tri = consts.tile([P, P], bf16, tag="tri", name="tri" + sfx)
            nc.sync.dma_start(bq_sb[:], bqs)
            nc.sync.dma_start(bk_sb[:], bks)
            nc.sync.dma_start(bv_sb[:], bvb)
            # triangle mask: keep col >= row, zero below
            nc.vector.memset(tri[:], 1.0)
            nc.gpsimd.affine_select(
                out=tri[:],
                in_=tri[:],
                compare_op=mybir.AluOpType.is_ge,
                fill=0.0,
                base=0,
                pattern=[[1, P]],
                channel_multiplier=-1,
            )

            # persistent activations: qT/kT rotate through a 2-deep pool
            # (pair m+1 reuses pair m-1's slot once its attention is done)
            qk = ctx0.enter_context(tc.tile_pool(name="qk" + sfx, bufs=2))
            qT2, kT2 = {}, {}
            v_sb = [
                vp.tile([P, HLOC, DK + 1], bf16, tag=f"v{t}", name=f"v{t}" + sfx)
                for t in range(NT)
            ]
            oTp = [
                oTp_pool.tile([P, S], f32r, tag=f"oTp{m}", name=f"oTp{m}" + sfx)
                for m in range(NPAIR)
            ]
            # output-projection weights tile; the DMA is issued after the
            # x/wv loads so it does not delay the first projections
            woT_sb = consts.tile([P, NPAIR, D], f32r, tag="woT", name="woT_sb" + sfx)

            with ExitStack() as ctxA:
                oTps = ctxA.enter_context(
                    tc.tile_pool(name="oTps" + sfx, bufs=3, space="PSUM")
                )
                stps = ctxA.enter_context(
                    tc.tile_pool(name="stps" + sfx, bufs=2, space="PSUM")
                )

                def normalize(h, c, oT_aug):
                    m, half = divmod(h, 2)
                    idx = h * NCH + c
                    t1 = norm_pool.tile([P, 512], f32, tag="t1", name=f"t1_{h}_{c}" + sfx)
                    nc.vector.tensor_copy(t1[64:65, :], oT_aug[64:65, :])
                    oT_un = norm_pool.tile(
                        [64, 512], f32, tag="oT_un", name=f"oTun_{h}_{c}" + sfx
                    )
                    nc.vector.tensor_copy(oT_un[:], oT_aug[0:64, :])
                    # denom row -> DRAM, spread over 64 partitions, DVE
                    # reciprocal, back to DRAM, broadcast to [64, 512]
                    nc.sync.dma_start(scr[idx : idx + 1, :], t1[64:65, :])
                    spread = norm_pool.tile(
                        [64, 8], f32, tag="spread", name=f"spread_{h}_{c}" + sfx
                    )
                    spread_src = bass.AP(
                        tensor=scr.tensor,
                        offset=scr.offset + idx * 512,
                        ap=[[8, 64], [1, 8]],
                    )
                    nc.sync.dma_start(spread[:], spread_src)
                    rspread = norm_pool.tile(
                        [64, 8], f32, tag="rspread", name=f"rspread_{h}_{c}" + sfx
                    )
                    nc.vector.reciprocal(rspread[:], spread[:])
                    nc.sync.dma_start(
                        bass.AP(
                            tensor=scr.tensor,
                            offset=scr.offset + (32 + idx) * 512,
                            ap=[[8, 64], [1, 8]],
                        ),
                        rspread[:],
                    )
                    rb = norm_pool.tile([64, 512], f32, tag="rb", name=f"rb_{h}_{c}" + sfx)
                    bcast_src = bass.AP(
                        tensor=scr.tensor,
                        offset=scr.offset + (32 + idx) * 512,
                        ap=[[0, 64], [1, 512]],
                    )
                    nc.sync.dma_start(rb[:], bcast_src)
                    if half == 0:
                        nc.vector.tensor_mul(
                            oTp[m][0:64, 512 * c : 512 * (c + 1)],
                            oT_un[:],
                            rb[:],
                        )
                    else:
                        stg = stage_pool.tile(
                            [64, 512], f32r, tag="stg", name=f"stg_{h}_{c}" + sfx
                        )
                        nc.vector.tensor_mul(stg[:], oT_un[:], rb[:])
                        nc.sync.dma_start(
                            oTp[m][64:P, 512 * c : 512 * (c + 1)], stg[:]
                        )

                def attention_pair(m):
                    """Heads 2m (qT/kT rows 0:64) and 2m+1 (rows 64:128),
                    c-outer / j-inner.  The two heads' score matmuls go to
                    disjoint PE row groups and run concurrently."""
                    ha, hb = 2 * m, 2 * m + 1
                    for c in range(NCH):
                        oT_a = oTps.tile(
                            [65, 512], f32, tag="oTaug", name=f"oTa{m}_{c}" + sfx
                        )
                        oT_b = oTps.tile(
                            [65, 512], f32, tag="oTaug", name=f"oTb{m}_{c}" + sfx
                        )
                        for j in range(4 * c + 4):
                            off = max(0, P * j - 512 * c)
                            w = 512 - off
                            st_ab = stps.tile(
                                [P, 1024], f32, tag="st", name=f"st{m}_{c}_{j}" + sfx
                            )
                            nc.tensor.matmul(
                                st_ab[:, off:512],
                                kT2[m][0:64, P * j : P * (j + 1)],
                                qT2[m][0:64, 512 * c + off : 512 * (c + 1)],
                                start=True,
                                stop=True,
                            )
                            nc.tensor.matmul(
                                st_ab[:, 512 + off : 1024],
                                kT2[m][64:P, P * j : P * (j + 1)],
                                qT2[m][64:P, 512 * c + off : 512 * (c + 1)],
                                start=True,
                                stop=True,
                            )
                            pexp2 = pexp_pool.tile(
                                [P, 2, 512], bf16, tag="pexp",
                                name=f"pexp{m}_{c}_{j}" + sfx,
                            )
                            nc.scalar.activation(
                                pexp2[:, :, 0:w],
                                st_ab[:].rearrange("p (two x) -> p two x", two=2)[
                                    :, :, off:512
                                ],
                                AF.Exp,
                            )
                            if j >= 4 * c:
                                # diagonal 128-block: strict lower triangle -> 0
                                nc.vector.tensor_mul(
                                    pexp2[:, :, 0:P],
                                    pexp2[:, :, 0:P],
                                    tri[:, None, :].to_broadcast((P, 2, P)),
                                )
                            nc.tensor.matmul(
                                oT_a[:, off:512],
                                v_sb[j][:, ha, :],
                                pexp2[:, 0, 0:w],
                                start=(j == 0),
                                stop=(j == 4 * c + 3),
                            )
                            nc.tensor.matmul(
                                oT_b[:, off:512],
                                v_sb[j][:, hb, :],
                                pexp2[:, 1, 0:w],
                                start=(j == 0),
                                stop=(j == 4 * c + 3),
                            )
                        normalize(ha, c, oT_a)
                        normalize(hb, c, oT_b)

                with ExitStack() as ctxP:
                    xpool = ctxP.enter_context(
                        tc.tile_pool(name="xpool" + sfx, bufs=1)
                    )
                    wvpool = ctxP.enter_context(
                        tc.tile_pool(name="wvpool" + sfx, bufs=1)
                    )
                    wqkpool = ctxP.enter_context(
                        tc.tile_pool(name="wqkpool" + sfx, bufs=4)
                    )
                    pps = ctxP.enter_context(
                        tc.tile_pool(name="pps" + sfx, bufs=1, space="PSUM")
                    )

                    wv_sb = wvpool.tile(
                        [P, IC, 512], bf16, tag="wv", name="wv_sb" + sfx
                    )
                    for ic in range(IC):
                        nc.sync.dma_start(
                            wv_sb[:, ic, :], wvt[P * ic : P * (ic + 1), :]
                        )
                    xT_sb = xpool.tile([P, IC, S], bf16, tag="xT", name="xT_sb" + sfx)
                    for ic in range(IC):
                        nc.sync.dma_start(
                            xT_sb[:, ic, :], xt[P * ic : P * (ic + 1), :]
                        )
                    for m in range(NPAIR):
                        nc.sync.dma_start(
                            woT_sb[:, m, :],
                            wot.rearrange("(m p) o -> p m o", p=P)[:, m, :],
                        )

                    # V projection (natural layout) + bias + ones column
                    for t in range(NT):
                        psv = stps.tile([P, 1024], f32, tag="st", name=f"psv{t}" + sfx)
                        for ic in range(IC):
                            nc.tensor.matmul(
                                psv[:, 0:512],
                                xT_sb[:, ic, P * t : P * (t + 1)],
                                wv_sb[:, ic, :],
                                start=(ic == 0),
                                stop=(ic == IC - 1),
                            )
                        nc.vector.tensor_add(
                            v_sb[t][:, :, 0:DK],
                            psv[:, 0:512].rearrange("p (h d) -> p h d", d=DK),
                            bv_sb[:].rearrange("p (h d) -> p h d", d=DK),
                        )
                        nc.vector.memset(v_sb[t][:, :, DK : DK + 1], 1.0)

                    def project_pair(m):
                        qT2[m] = qk.tile(
                            [P, S], f32r, tag="qT", name=f"qT{m}" + sfx
                        )
                        kT2[m] = qk.tile(
                            [P, S], f32r, tag="kT", name=f"kT{m}" + sfx
                        )
                        wq_m = wqkpool.tile(
                            [P, IC, P], bf16, tag="wqk", name=f"wq{m}" + sfx
                        )
                        wk_m = wqkpool.tile(
                            [P, IC, P], bf16, tag="wqk", name=f"wk{m}" + sfx
                        )
                        nc.sync.dma_start(
                            wq_m[:],
                            wqt.rearrange("(ic p) o -> p ic o", p=P)[
                                :, :, P * m : P * (m + 1)
                            ],
                        )
                        nc.sync.dma_start(
                            wk_m[:],
                            wkt.rearrange("(ic p) o -> p ic o", p=P)[
                                :, :, P * m : P * (m + 1)
                            ],
                        )
                        for c in range(NCH):
                            psq = pps.tile(
                                [P, 512], f32, tag="pps", name=f"psq{m}_{c}" + sfx
                            )
                            for ic in range(IC):
                                nc.tensor.matmul(
                                    psq[:],
                                    wq_m[:, ic, :],
                                    xT_sb[:, ic, 512 * c : 512 * (c + 1)],
                                    start=(ic == 0),
                                    stop=(ic == IC - 1),
                                )
                            nc.scalar.activation(
                                qT2[m][:, 512 * c : 512 * (c + 1)],
                                psq[:],
                                AF.Identity,
                                bias=bq_sb[:, m : m + 1],
                                scale=0.125,
                            )
                            psk = pps.tile(
                                [P, 512], f32, tag="pps", name=f"psk{m}_{c}" + sfx
                            )
                            for ic in range(IC):
                                nc.tensor.matmul(
                                    psk[:],
                                    wk_m[:, ic, :],
                                    xT_sb[:, ic, 512 * c : 512 * (c + 1)],
                                    start=(ic == 0),
                                    stop=(ic == IC - 1),
                                )
                            nc.scalar.activation(
                                kT2[m][:, 512 * c : 512 * (c + 1)],
                                psk[:],
                                AF.Identity,
                                bias=bk_sb[:, m : m + 1],
                                scale=1.0,
                            )

                    project_pair(0)
                    for m in range(NPAIR):
                        attention_pair(m)
                        if m + 1 < NPAIR:
                            project_pair(m + 1)

            # output projection
            fst_pool = tc.tile_pool(name="fst" + sfx, bufs=4)
            fin_ps = tc.tile_pool(name="finps" + sfx, bufs=6, space="PSUM")
            with fst_pool as fsp, fin_ps as fps:
                for t in range(NT):
                    for n in range(2):
                        psf = fps.tile(
                            [P, 512], f32, tag="fin", name=f"fin{t}_{n}" + sfx
                        )
                        for m in range(NPAIR):
                            nc.tensor.matmul(
                                psf[:],
                                oTp[m][:, P * t : P * (t + 1)],
                                woT_sb[:, m, 512 * n : 512 * (n + 1)],
                                start=(m == 0),
                                stop=(m == NPAIR - 1),
                            )
                        fstg = fsp.tile(
                            [P, 512], f32, tag="fstg", name=f"fstg{t}_{n}" + sfx
                        )
                        nc.any.tensor_copy(fstg[:], psf[:])
                        nc.sync.dma_start(
                            out[P * t : P * (t + 1), 512 * n : 512 * (n + 1)],
                            fstg[:],
                        )

    nc.compile()
    return nc


def get_program(reps=1):
    if reps not in _PROGRAM_CACHE:
        _PROGRAM_CACHE[reps] = build_program(reps)
    return _PROGRAM_CACHE[reps]


def make_in_maps(x, w_q, b_q, w_k, b_k, w_v, b_v, w_o):
    bf = ml_dtypes.bfloat16
    x = np.asarray(x, np.float32)
    w_q = np.asarray(w_q, np.float32)
    w_k = np.asarray(w_k, np.float32)
    w_v = np.asarray(w_v, np.float32)
    w_o = np.asarray(w_o, np.float32)
    b_q = np.asarray(b_q, np.float32)
    b_k = np.asarray(b_k, np.float32)
    b_v = np.asarray(b_v, np.float32)
    in_maps = []
    for core in range(8):
        b, g = divmod(core, 2)
        sl = slice(512 * g, 512 * (g + 1))
        in_maps.append(
            {
                "xt": np.ascontiguousarray(x[b].T).astype(bf),
                "wqt": np.ascontiguousarray(w_q[sl].T).astype(bf),
                "wkt": np.ascontiguousarray(w_k[sl].T).astype(bf),
                "wvt": np.ascontiguousarray(w_v[sl].T).astype(bf),
                "wot": np.ascontiguousarray(w_o[:, sl].T),
                "bqs": np.ascontiguousarray((b_q[sl] * 0.125).reshape(NPAIR, P).T),
                "bks": np.ascontiguousarray(b_k[sl].reshape(NPAIR, P).T),
                "bvb": np.ascontiguousarray(np.tile(b_v[sl][None, :], (P, 1))),
            }
        )
    return in_maps


def kernel(x, mask, w_q, b_q, w_k, b_k, w_v, b_v, w_o, b_o):
    from concourse.bass_utils import run_bass_kernel_spmd

    nc = get_program()
    in_maps = make_in_maps(x, w_q, b_q, w_k, b_k, w_v, b_v, w_o)
    res = run_bass_kernel_spmd(nc, in_maps, core_ids=list(range(8)), trace=False)
    b_o = np.asarray(b_o, np.float32)
    outs = []
    for b in range(4):
        outs.append(
            res.results[2 * b]["out"] + res.results[2 * b + 1]["out"] + b_o[None, :]
        )
    return np.stack(outs).astype(np.float32)
